# revision 24
# baseline (speedup 1.0000x reference)
"""MoE (32 experts, top-4, T=64, H=2048, I=1408) — expert-parallel Bass kernel
for 8 trn2 NeuronCores.

Strategy (hardcoded, matches the expert-parallel sharding hint):
  - Each core owns 4 experts; the host ships weight shards pre-transposed and
    pre-tiled into the exact SBUF layout ([128 partitions, chunk, free]) in
    bf16, so every DMA is a long contiguous run per partition.
  - x (as x.T, bf16) and router logits are replicated; logits columns are
    permuted per-core so the local experts are always columns 0..3 (keeps
    the SPMD program identical across cores).
  - On-device: top-4 + softmax routing weights; per local expert a dense FFN
    with gate/up computed transposed ([inter, tok], weight-stationary bf16
    matmuls, fp32 PSUM accumulate) so the down-projection needs no on-chip
    transpose; the down output ([tok, hidden]) is scaled by the per-token
    routing weight and accumulated across local experts on the DVE; finally a
    ReduceScatter(add) over the 8 cores.
  - Each core returns its 8-token output shard; the host concatenates.
"""

import sys

sys.path.insert(0, "/opt/trn_rl_repo")

import ml_dtypes
import numpy as np

import concourse.bass as bass
import concourse.tile as tile
from concourse import bacc, bass_utils, mybir

T = 64
H = 2048
I = 1408
E = 32
TOPK = 4
NCORES = 8
EPC = E // NCORES  # experts per core
HC = H // 128  # 16 h-chunks
IC = I // 128  # 11 i-chunks
TSH = T // NCORES  # tokens per output shard

GUD = 8  # h-chunks per gate/up DMA
WD_CH = [(0, 4), (4, 4), (8, 3)]  # (start,count) i-chunk groups per down DMA

f32 = mybir.dt.float32
bf16 = mybir.dt.bfloat16
Alu = mybir.AluOpType
Act = mybir.ActivationFunctionType

_BF16 = np.dtype(ml_dtypes.bfloat16)


def _build_program():
    nc = bacc.Bacc(
        "TRN2",
        target_bir_lowering=False,
        debug=False,
        enable_asserts=False,
        num_devices=NCORES,
    )

    xT_d = nc.dram_tensor("xT", [128, HC, T], bf16, kind="ExternalInput")
    lg_d = nc.dram_tensor("logits", [T, E], f32, kind="ExternalInput")
    wgT_d = nc.dram_tensor("wgT", [EPC, 128, HC, I], bf16, kind="ExternalInput")
    wuT_d = nc.dram_tensor("wuT", [EPC, 128, HC, I], bf16, kind="ExternalInput")
    wdT_d = nc.dram_tensor("wdT", [EPC, 128, IC, H], bf16, kind="ExternalInput")
    out_d = nc.dram_tensor("out", [TSH, H], f32, kind="ExternalOutput")

    with tile.TileContext(nc) as tc:
        _kernel_body(tc, xT_d, lg_d, wgT_d, wuT_d, wdT_d, out_d)
    nc.compile()
    return nc


def _kernel_body(tc, xT_d, lg_d, wgT_d, wuT_d, wdT_d, out_d):
    nc = tc.nc
    from contextlib import ExitStack

    ctx = ExitStack()
    with ctx:
        const = ctx.enter_context(tc.tile_pool(name="const", bufs=1))
        small = ctx.enter_context(tc.tile_pool(name="small", bufs=2))
        wg_pool = ctx.enter_context(tc.tile_pool(name="wg", bufs=2))
        wu_pool = ctx.enter_context(tc.tile_pool(name="wu", bufs=2))
        wd_pool = ctx.enter_context(tc.tile_pool(name="wd", bufs=4))
        act_pool = ctx.enter_context(tc.tile_pool(name="act", bufs=2))
        psg = ctx.enter_context(tc.tile_pool(name="psg", bufs=1, space="PSUM"))
        psu = ctx.enter_context(tc.tile_pool(name="psu", bufs=1, space="PSUM"))
        psd = ctx.enter_context(tc.tile_pool(name="psd", bufs=1, space="PSUM"))
        dram = ctx.enter_context(tc.tile_pool(name="dram", bufs=1, space="DRAM"))

        # ---- x (transposed, bf16) ----
        xt = const.tile([128, HC, T], bf16)  # x.T as [h_par, h_chunk, tok]
        nc.sync.dma_start(xt[:], xT_d.ap())

        # ---- routing: top-4 + softmax over selected logits ----
        lg = const.tile([T, E], f32)
        nc.sync.dma_start(lg[:], lg_d.ap())

        work = small.tile([T, E], f32)
        nc.vector.tensor_copy(work[:], lg[:])
        negm0 = const.tile([T, 1], f32)
        mlast = const.tile([T, 1], f32)
        for k in range(TOPK):
            m = small.tile([T, 1], f32, tag="mk")
            nc.vector.tensor_reduce(m[:], work[:], axis=mybir.AxisListType.X, op=Alu.max)
            if k == 0:
                nc.vector.tensor_scalar_mul(negm0[:], m[:], -1.0)
            if k == TOPK - 1:
                nc.vector.tensor_copy(mlast[:], m[:])
            else:
                eq = small.tile([T, E], f32, tag="eq")
                nc.vector.tensor_scalar(eq[:], work[:], m[:], None, op0=Alu.is_equal)
                nc.vector.tensor_scalar(eq[:], eq[:], 1e30, None, op0=Alu.mult)
                nc.vector.tensor_tensor(work[:], work[:], eq[:], op=Alu.subtract)

        sel = small.tile([T, E], f32)
        nc.vector.tensor_scalar(sel[:], lg[:], mlast[:], None, op0=Alu.is_ge)
        ex = small.tile([T, E], f32)
        nc.scalar.activation(ex[:], lg[:], func=Act.Exp, bias=negm0[:], scale=1.0)
        nc.vector.tensor_tensor(ex[:], ex[:], sel[:], op=Alu.mult)
        den = small.tile([T, 1], f32)
        nc.vector.reduce_sum(den[:], ex[:], axis=mybir.AxisListType.X)
        rec = small.tile([T, 1], f32)
        nc.vector.reciprocal(rec[:], den[:])
        G = const.tile([T, E], f32)  # routing weights, local experts = cols 0..EPC-1
        nc.vector.tensor_scalar(G[:], ex[:], rec[:], None, op0=Alu.mult)

        # ---- main expert loop ----
        out_acc = const.tile([T, H], f32)

        for e in range(EPC):
            gateT_ps = psg.tile([128, IC * T], f32, tag="g")
            upT_ps = psu.tile([128, IC * T], f32, tag="u")
            down_ps = psd.tile([T, H], f32, tag="down")
            siluT = act_pool.tile([128, IC * T], f32, tag="silu")
            mixT = act_pool.tile([128, IC * T], bf16, tag="mixT")

            # prefetch the down-projection weights early (own queue)
            wdts = []
            for (c0, cn) in WD_CH:
                wdt = wd_pool.tile([128, 4, H], bf16, tag="wd")
                nc.gpsimd.dma_start(wdt[:, :cn, :], wdT_d.ap()[e, :, c0 : c0 + cn, :])
                wdts.append(wdt)

            # gate/up: h-major streaming, weight-stationary matmuls
            for d in range(HC // GUD):
                wgt = wg_pool.tile([128, GUD, I], bf16, tag="wg")
                nc.sync.dma_start(wgt[:], wgT_d.ap()[e, :, GUD * d : GUD * (d + 1), :])
                wut = wu_pool.tile([128, GUD, I], bf16, tag="wu")
                nc.scalar.dma_start(wut[:], wuT_d.ap()[e, :, GUD * d : GUD * (d + 1), :])
                for a in range(GUD):
                    hc = GUD * d + a
                    xmv = xt[:, hc, :]
                    for it in range(IC):
                        first = hc == 0 and it in (0, 8)
                        nc.tensor.matmul(
                            gateT_ps[:, T * it : T * (it + 1)],
                            wgt[:, a, 128 * it : 128 * (it + 1)],
                            xmv,
                            start=first,
                            stop=(hc == HC - 1),
                        )
                        nc.tensor.matmul(
                            upT_ps[:, T * it : T * (it + 1)],
                            wut[:, a, 128 * it : 128 * (it + 1)],
                            xmv,
                            start=first,
                            stop=(hc == HC - 1),
                        )

            # per chunk: mixT = silu(gateT)*upT (bf16), then its down matmuls
            for it in range(IC):
                sl = slice(T * it, T * (it + 1))
                nc.scalar.activation(siluT[:, sl], gateT_ps[:, sl], func=Act.Silu)
                nc.vector.tensor_tensor(mixT[:, sl], siluT[:, sl], upT_ps[:, sl], op=Alu.mult)
                wdt = wdts[it // 4]
                for b in range(H // 512):
                    nc.tensor.matmul(
                        down_ps[:, 512 * b : 512 * (b + 1)],
                        mixT[:, sl],
                        wdt[:, it % 4, 512 * b : 512 * (b + 1)],
                        start=(it == 0),
                        stop=(it == IC - 1),
                    )

            # out_acc += G[:, e] * down   (per-token routing weight)
            if e == 0:
                nc.vector.tensor_scalar(
                    out_acc[:], down_ps[:], G[:, e : e + 1], None, op0=Alu.mult
                )
            else:
                nc.vector.scalar_tensor_tensor(
                    out_acc[:], down_ps[:], G[:, e : e + 1], out_acc[:],
                    op0=Alu.mult, op1=Alu.add,
                )

        # ---- reduce-scatter over the 8 cores, emit this core's token shard ----
        cc_in = dram.tile([T, H], f32)
        cc_out = dram.tile([TSH, H], f32)
        nc.sync.dma_start(cc_in[:], out_acc[:])
        nc.gpsimd.collective_compute(
            "ReduceScatter",
            Alu.add,
            replica_groups=[list(range(NCORES))],
            ins=[cc_in.opt()],
            outs=[cc_out.opt()],
        )
        nc.sync.dma_start(out_d.ap(), cc_out[:])


_PROGRAM = None


def _get_program():
    global _PROGRAM
    if _PROGRAM is None:
        _PROGRAM = _build_program()
    return _PROGRAM


def _sbuf_layout(w, free):
    """[n, free_out, contract] expert weights -> [n, 128, chunks, free] bf16:
    transposed so the contraction dim is on partitions, tiled so each
    partition's data per chunk-group is one long contiguous DRAM run."""
    n, fo, contract = w.shape
    chunks = contract // 128
    a = w.transpose(0, 2, 1).reshape(n, chunks, 128, fo).transpose(0, 2, 1, 3)
    return np.ascontiguousarray(a.astype(_BF16))


def _make_in_maps(x, router_logits, w_gate, w_up, w_down):
    xT = np.ascontiguousarray(
        np.asarray(x, np.float32).T.reshape(HC, 128, T).transpose(1, 0, 2).astype(_BF16)
    )
    in_maps = []
    for c in range(NCORES):
        lo, hi = c * EPC, (c + 1) * EPC
        perm = list(range(lo, hi)) + [i for i in range(E) if not (lo <= i < hi)]
        lg_c = np.ascontiguousarray(router_logits[:, perm].astype(np.float32, copy=False))
        in_maps.append(
            {
                "xT": xT,
                "logits": lg_c,
                "wgT": _sbuf_layout(w_gate[lo:hi], I),
                "wuT": _sbuf_layout(w_up[lo:hi], I),
                "wdT": _sbuf_layout(w_down[lo:hi], H),
            }
        )
    return in_maps


def kernel(x, router_logits, w_gate, w_up, w_down, _trace=False, _results_out=None):
    x = np.asarray(x, dtype=np.float32)
    router_logits = np.asarray(router_logits, dtype=np.float32)
    w_gate = np.asarray(w_gate, dtype=np.float32)
    w_up = np.asarray(w_up, dtype=np.float32)
    w_down = np.asarray(w_down, dtype=np.float32)

    nc = _get_program()
    in_maps = _make_in_maps(x, router_logits, w_gate, w_up, w_down)
    res = bass_utils.run_bass_kernel_spmd(
        nc, in_maps, core_ids=list(range(NCORES)), trace=_trace
    )
    if _results_out is not None:
        _results_out.append(res)
    shards = [res.results[c]["out"] for c in range(NCORES)]
    out = np.concatenate(shards, axis=0)  # [T, H]
    return out[:, None, :].astype(np.float32)


# revision 26
# speedup vs baseline: 1.0505x; 1.0505x over previous
"""MoE (32 experts, top-4, T=64, H=2048, I=1408) — expert-parallel Bass kernel
for 8 trn2 NeuronCores.

Strategy (hardcoded, matches the expert-parallel sharding hint):
  - Each core owns 4 experts; the host ships weight shards pre-transposed and
    pre-tiled into the exact SBUF layout ([128 partitions, chunk, free]) in
    bf16, so every DMA is a long contiguous run per partition.
  - x (as x.T, bf16) and router logits are replicated; logits columns are
    permuted per-core so the local experts are always columns 0..3 (keeps
    the SPMD program identical across cores).
  - On-device: top-4 + softmax routing weights; per local expert a dense FFN
    with gate/up computed transposed ([inter, tok], weight-stationary bf16
    matmuls, fp32 PSUM accumulate) so the down-projection needs no on-chip
    transpose; the down output ([tok, hidden]) is scaled by the per-token
    routing weight and accumulated across local experts on the DVE; finally a
    ReduceScatter(add) over the 8 cores.
  - Each core returns its 8-token output shard; the host concatenates.
"""

import sys

sys.path.insert(0, "/opt/trn_rl_repo")

import ml_dtypes
import numpy as np

import concourse.bass as bass
import concourse.tile as tile
from concourse import bacc, bass_utils, mybir

T = 64
H = 2048
I = 1408
E = 32
TOPK = 4
NCORES = 8
EPC = E // NCORES  # experts per core
HC = H // 128  # 16 h-chunks
IC = I // 128  # 11 i-chunks
TSH = T // NCORES  # tokens per output shard

GUD = 8  # h-chunks per gate/up DMA
WD_CH = [(0, 4), (4, 4), (8, 3)]  # (start,count) i-chunk groups per down DMA

f32 = mybir.dt.float32
bf16 = mybir.dt.bfloat16
Alu = mybir.AluOpType
Act = mybir.ActivationFunctionType

_BF16 = np.dtype(ml_dtypes.bfloat16)


def _build_program():
    nc = bacc.Bacc(
        "TRN2",
        target_bir_lowering=False,
        debug=False,
        enable_asserts=False,
        num_devices=NCORES,
    )

    xT_d = nc.dram_tensor("xT", [128, HC, T], bf16, kind="ExternalInput")
    lg_d = nc.dram_tensor("logits", [T, E], f32, kind="ExternalInput")
    wgT_d = nc.dram_tensor("wgT", [EPC, 128, HC, I], bf16, kind="ExternalInput")
    wuT_d = nc.dram_tensor("wuT", [EPC, 128, HC, I], bf16, kind="ExternalInput")
    wdT_d = nc.dram_tensor("wdT", [EPC, 128, IC, H], bf16, kind="ExternalInput")
    out_d = nc.dram_tensor("out", [TSH, H], f32, kind="ExternalOutput")

    with tile.TileContext(nc) as tc:
        _kernel_body(tc, xT_d, lg_d, wgT_d, wuT_d, wdT_d, out_d)
    nc.compile()
    return nc


def _kernel_body(tc, xT_d, lg_d, wgT_d, wuT_d, wdT_d, out_d):
    nc = tc.nc
    from contextlib import ExitStack

    ctx = ExitStack()
    with ctx:
        const = ctx.enter_context(tc.tile_pool(name="const", bufs=1))
        small = ctx.enter_context(tc.tile_pool(name="small", bufs=2))
        wg_pool = ctx.enter_context(tc.tile_pool(name="wg", bufs=2))
        wu_pool = ctx.enter_context(tc.tile_pool(name="wu", bufs=2))
        wd_pool = ctx.enter_context(tc.tile_pool(name="wd", bufs=3))
        act_pool = ctx.enter_context(tc.tile_pool(name="act", bufs=2))
        psg = ctx.enter_context(tc.tile_pool(name="psg", bufs=1, space="PSUM"))
        psu = ctx.enter_context(tc.tile_pool(name="psu", bufs=1, space="PSUM"))
        psd = ctx.enter_context(tc.tile_pool(name="psd", bufs=1, space="PSUM"))
        dram = ctx.enter_context(tc.tile_pool(name="dram", bufs=1, space="DRAM"))

        # ---- x (transposed, bf16) ----
        xt = const.tile([128, HC, T], bf16)  # x.T as [h_par, h_chunk, tok]
        nc.sync.dma_start(xt[:], xT_d.ap())

        # ---- routing: top-4 + softmax over selected logits ----
        lg = const.tile([T, E], f32)
        nc.sync.dma_start(lg[:], lg_d.ap())

        work = small.tile([T, E], f32)
        nc.vector.tensor_copy(work[:], lg[:])
        negm0 = const.tile([T, 1], f32)
        mlast = const.tile([T, 1], f32)
        for k in range(TOPK):
            m = small.tile([T, 1], f32, tag="mk")
            nc.vector.tensor_reduce(m[:], work[:], axis=mybir.AxisListType.X, op=Alu.max)
            if k == 0:
                nc.vector.tensor_scalar_mul(negm0[:], m[:], -1.0)
            if k == TOPK - 1:
                nc.vector.tensor_copy(mlast[:], m[:])
            else:
                eq = small.tile([T, E], f32, tag="eq")
                nc.vector.tensor_scalar(eq[:], work[:], m[:], None, op0=Alu.is_equal)
                nc.vector.tensor_scalar(eq[:], eq[:], 1e30, None, op0=Alu.mult)
                nc.vector.tensor_tensor(work[:], work[:], eq[:], op=Alu.subtract)

        sel = small.tile([T, E], f32)
        nc.vector.tensor_scalar(sel[:], lg[:], mlast[:], None, op0=Alu.is_ge)
        ex = small.tile([T, E], f32)
        nc.scalar.activation(ex[:], lg[:], func=Act.Exp, bias=negm0[:], scale=1.0)
        nc.vector.tensor_tensor(ex[:], ex[:], sel[:], op=Alu.mult)
        den = small.tile([T, 1], f32)
        nc.vector.reduce_sum(den[:], ex[:], axis=mybir.AxisListType.X)
        rec = small.tile([T, 1], f32)
        nc.vector.reciprocal(rec[:], den[:])
        G = const.tile([T, E], f32)  # routing weights, local experts = cols 0..EPC-1
        nc.vector.tensor_scalar(G[:], ex[:], rec[:], None, op0=Alu.mult)

        # ---- main expert loop ----
        out_acc = const.tile([T, H], f32)

        for e in range(EPC):
            last = e == EPC - 1
            gateT_ps = psg.tile([128, IC * T], f32, tag="g")
            upT_ps = psu.tile([128, IC * T], f32, tag="u")
            down_ps = psd.tile([T, H], f32, tag="down")
            siluT = act_pool.tile([128, IC * T], f32, tag="silu")
            mixT = act_pool.tile([128, IC * T], bf16, tag="mixT")

            # prefetch the down-projection weights early (own queue)
            wdts = []
            for (c0, cn) in WD_CH:
                wdt = wd_pool.tile([128, 4, H], bf16, tag="wd")
                nc.gpsimd.dma_start(wdt[:, :cn, :], wdT_d.ap()[e, :, c0 : c0 + cn, :])
                wdts.append(wdt)

            def mix_and_down(it):
                sl = slice(T * it, T * (it + 1))
                nc.scalar.activation(siluT[:, sl], gateT_ps[:, sl], func=Act.Silu)
                nc.vector.tensor_tensor(mixT[:, sl], siluT[:, sl], upT_ps[:, sl], op=Alu.mult)
                wdt = wdts[it // 4]
                for b in range(H // 512):
                    nc.tensor.matmul(
                        down_ps[:, 512 * b : 512 * (b + 1)],
                        mixT[:, sl],
                        wdt[:, it % 4, 512 * b : 512 * (b + 1)],
                        start=(it == 0),
                        stop=(it == IC - 1),
                    )

            if not last:
                # h-major streaming: big contiguous weight chunks
                for d in range(HC // GUD):
                    wgt = wg_pool.tile([128, GUD, I], bf16, tag="wg")
                    nc.sync.dma_start(wgt[:], wgT_d.ap()[e, :, GUD * d : GUD * (d + 1), :])
                    wut = wu_pool.tile([128, GUD, I], bf16, tag="wu")
                    nc.scalar.dma_start(wut[:], wuT_d.ap()[e, :, GUD * d : GUD * (d + 1), :])
                    for a in range(GUD):
                        hc = GUD * d + a
                        xmv = xt[:, hc, :]
                        for it in range(IC):
                            first = hc == 0 and it in (0, 8)
                            nc.tensor.matmul(
                                gateT_ps[:, T * it : T * (it + 1)],
                                wgt[:, a, 128 * it : 128 * (it + 1)],
                                xmv,
                                start=first,
                                stop=(hc == HC - 1),
                            )
                            nc.tensor.matmul(
                                upT_ps[:, T * it : T * (it + 1)],
                                wut[:, a, 128 * it : 128 * (it + 1)],
                                xmv,
                                start=first,
                                stop=(hc == HC - 1),
                            )
                for it in range(IC):
                    mix_and_down(it)
            else:
                # last expert: i-grouped so early chunks finish (and their
                # down matmuls run) while later groups are still streaming in
                for (c0, cn) in WD_CH:
                    iw = 128 * cn
                    wgt = wg_pool.tile([128, HC, iw], bf16, tag="wg")
                    nc.sync.dma_start(
                        wgt[:], wgT_d.ap()[e, :, :, 128 * c0 : 128 * c0 + iw]
                    )
                    wut = wu_pool.tile([128, HC, iw], bf16, tag="wu")
                    nc.scalar.dma_start(
                        wut[:], wuT_d.ap()[e, :, :, 128 * c0 : 128 * c0 + iw]
                    )
                    for hc in range(HC):
                        xmv = xt[:, hc, :]
                        for a in range(cn):
                            it = c0 + a
                            first = hc == 0 and it in (0, 8)
                            nc.tensor.matmul(
                                gateT_ps[:, T * it : T * (it + 1)],
                                wgt[:, hc, 128 * a : 128 * (a + 1)],
                                xmv,
                                start=first,
                                stop=(hc == HC - 1),
                            )
                            nc.tensor.matmul(
                                upT_ps[:, T * it : T * (it + 1)],
                                wut[:, hc, 128 * a : 128 * (a + 1)],
                                xmv,
                                start=first,
                                stop=(hc == HC - 1),
                            )
                    for a in range(cn):
                        mix_and_down(c0 + a)

            # out_acc += G[:, e] * down   (per-token routing weight)
            if e == 0:
                nc.vector.tensor_scalar(
                    out_acc[:], down_ps[:], G[:, e : e + 1], None, op0=Alu.mult
                )
            else:
                nc.vector.scalar_tensor_tensor(
                    out_acc[:], down_ps[:], G[:, e : e + 1], out_acc[:],
                    op0=Alu.mult, op1=Alu.add,
                )

        # ---- reduce-scatter over the 8 cores, emit this core's token shard ----
        cc_in = dram.tile([T, H], f32)
        cc_out = dram.tile([TSH, H], f32)
        nc.sync.dma_start(cc_in[:], out_acc[:])
        nc.gpsimd.collective_compute(
            "ReduceScatter",
            Alu.add,
            replica_groups=[list(range(NCORES))],
            ins=[cc_in.opt()],
            outs=[cc_out.opt()],
        )
        nc.sync.dma_start(out_d.ap(), cc_out[:])


_PROGRAM = None


def _get_program():
    global _PROGRAM
    if _PROGRAM is None:
        _PROGRAM = _build_program()
    return _PROGRAM


def _sbuf_layout(w, free):
    """[n, free_out, contract] expert weights -> [n, 128, chunks, free] bf16:
    transposed so the contraction dim is on partitions, tiled so each
    partition's data per chunk-group is one long contiguous DRAM run."""
    n, fo, contract = w.shape
    chunks = contract // 128
    a = w.transpose(0, 2, 1).reshape(n, chunks, 128, fo).transpose(0, 2, 1, 3)
    return np.ascontiguousarray(a.astype(_BF16))


def _make_in_maps(x, router_logits, w_gate, w_up, w_down):
    xT = np.ascontiguousarray(
        np.asarray(x, np.float32).T.reshape(HC, 128, T).transpose(1, 0, 2).astype(_BF16)
    )
    in_maps = []
    for c in range(NCORES):
        lo, hi = c * EPC, (c + 1) * EPC
        perm = list(range(lo, hi)) + [i for i in range(E) if not (lo <= i < hi)]
        lg_c = np.ascontiguousarray(router_logits[:, perm].astype(np.float32, copy=False))
        in_maps.append(
            {
                "xT": xT,
                "logits": lg_c,
                "wgT": _sbuf_layout(w_gate[lo:hi], I),
                "wuT": _sbuf_layout(w_up[lo:hi], I),
                "wdT": _sbuf_layout(w_down[lo:hi], H),
            }
        )
    return in_maps


def kernel(x, router_logits, w_gate, w_up, w_down, _trace=False, _results_out=None):
    x = np.asarray(x, dtype=np.float32)
    router_logits = np.asarray(router_logits, dtype=np.float32)
    w_gate = np.asarray(w_gate, dtype=np.float32)
    w_up = np.asarray(w_up, dtype=np.float32)
    w_down = np.asarray(w_down, dtype=np.float32)

    nc = _get_program()
    in_maps = _make_in_maps(x, router_logits, w_gate, w_up, w_down)
    res = bass_utils.run_bass_kernel_spmd(
        nc, in_maps, core_ids=list(range(NCORES)), trace=_trace
    )
    if _results_out is not None:
        _results_out.append(res)
    shards = [res.results[c]["out"] for c in range(NCORES)]
    out = np.concatenate(shards, axis=0)  # [T, H]
    return out[:, None, :].astype(np.float32)
